# revision 2
# baseline (speedup 1.0000x reference)
"""Trainium2 Bass kernel for nn_Agent (5-GRU actor-critic encoder + value MLP).

v4 = the tuned baseline compute graph + ONE packed input tensor.

The per-dispatch runtime overhead through the PJRT path is ~34us per bound
input tensor; the previous 25-input layout spent more wall time on buffer
binding than on the kernel itself. All inputs (pre-transposed activations +
pre-arranged weights) are packed host-side into a single flat bf16 tensor
with fixed offsets; a second small fp32 tensor exists only in the (unused in
practice) nonzero-bias fallback.

Compute structure (unchanged from the tuned baseline): data-parallel over
batch (256 per core), feature-on-partitions / batch-on-free SBUF layout,
the three small GRUs packed block-diagonally into one 128-partition lane,
pair-batched small-lane r/z input projections (N=512), double-buffered
per-step oa r/z PSUM tiles, gate math split across DVE/GpSimd with the
xn + r*hn add accumulated on the TensorEngine via an identity matmul.
Zero-bias fast path: tanh reads PSUM without bias and t1 is a plain
tensor-tensor multiply.
"""

import os
import sys
import time

import numpy as np

for _p in ("/opt/trn_rl_repo", "/root/.axon_site/_ro/trn_rl_repo"):
    if _p not in sys.path and os.path.isdir(_p):
        sys.path.insert(0, _p)

import concourse.bass as bass  # noqa: E402
import concourse.mybir as mybir  # noqa: E402
import concourse.tile as tile  # noqa: E402
from concourse import bacc  # noqa: E402

F32 = mybir.dt.float32
BF16 = mybir.dt.bfloat16
AFT = mybir.ActivationFunctionType
OP = mybir.AluOpType

T, B, F, A = 128, 2048, 256, 64
NCORES = 8
BL = B // NCORES  # 256 batch per core

_GATE = 128

_WSPECS = [
    ("woa_hh", (128, 384)),
    ("woa_obs", (128, 2, 384)),
    ("woa_act", (64, 384)),
    ("wmx_hh", (128, 384)),
    ("wmx_ih", (128, 384)),
    ("wsm_hh", (128, 384)),
    ("wsm_obs", (128, 2, 384)),
    ("wsm_pr", (65, 384)),
    ("w1t", (128, 2, 256)),
    ("w2t", (128, 2, 1)),
    ("wident", (128, 128)),
]

_BSPECS = [
    ("bnh_oa", (128, 1)), ("bnh_sm", (128, 1)), ("bnh_mx", (128, 1)),
    ("bni_oa", (128, 1)), ("bni_sm", (128, 1)), ("bni_mx", (128, 1)),
    ("brz_oa", (128, 2)), ("brz_sm", (128, 2)), ("brz_mx", (128, 2)),
    ("b1", (128, 2)), ("b2", (1, 1)),
]


def _layout(T_steps: int):
    off = {}
    o = 0
    off["obs"] = o
    o += T_steps * F * BL
    off["act"] = o
    o += T_steps * A * BL
    off["pr"] = o
    o += T_steps * (A + 1) * BL
    for name, shape in _WSPECS:
        off[name] = o
        o += int(np.prod(shape))
    off["_total"] = o
    return off


def _blayout():
    off = {}
    o = 0
    for name, shape in _BSPECS:
        off[name] = o
        o += int(np.prod(shape))
    off["_total"] = o
    return off


def _build(T_steps: int, use_bias: bool):
    nc = bacc.Bacc("TRN2", target_bir_lowering=False, debug=False,
                   num_devices=1)
    lay = _layout(T_steps)
    flat = nc.dram_tensor("flat", [lay["_total"]], BF16, kind="ExternalInput")
    if use_bias:
        blay = _blayout()
        extras = nc.dram_tensor("extras", [blay["_total"]], F32,
                                kind="ExternalInput")
    val = nc.dram_tensor("val", [1, BL], F32, kind="ExternalOutput")

    obs3 = flat[lay["obs"]:lay["obs"] + T_steps * F * BL].rearrange(
        "(t p b) -> t p b", p=F, b=BL)
    act3 = flat[lay["act"]:lay["act"] + T_steps * A * BL].rearrange(
        "(t p b) -> t p b", p=A, b=BL)
    pr3 = flat[lay["pr"]:lay["pr"] + T_steps * (A + 1) * BL].rearrange(
        "(t p b) -> t p b", p=A + 1, b=BL)

    def wsrc(name):
        shape = dict(_WSPECS)[name]
        sz = int(np.prod(shape))
        ap = flat[lay[name]:lay[name] + sz]
        if len(shape) == 2:
            return ap.rearrange("(p m) -> p m", p=shape[0])
        return ap.rearrange("(p a m) -> p a m", p=shape[0], a=shape[1])

    with tile.TileContext(nc) as tc:
        with (
            tc.tile_pool(name="const", bufs=1) as cp,
            tc.tile_pool(name="io", bufs=4) as iop,
            tc.tile_pool(name="psum_pair", bufs=1, space="PSUM") as ppp,
            tc.tile_pool(name="psum", bufs=1, space="PSUM") as pp,
            tc.tile_pool(name="psum_nh2", bufs=2, space="PSUM") as pp2,
            tc.tile_pool(name="tmp", bufs=3) as tp,
            tc.tile_pool(name="state", bufs=4) as hp,
        ):
            # ---- prefetch first input chunk before weight loads ----
            nch0 = min(8, T_steps)
            obs_c0 = [iop.tile([128, nch0, BL], BF16, tag=f"obs{k}",
                               name=f"obs{k}_pre") for k in range(2)]
            for k in range(2):
                nc.sync.dma_start(
                    obs_c0[k],
                    obs3[0:nch0, k * 128:(k + 1) * 128].rearrange(
                        "t p b -> p t b"))
            act_c0 = iop.tile([64, nch0, BL], BF16, tag="act", name="act_pre")
            nc.sync.dma_start(act_c0, act3[0:nch0].rearrange("t p b -> p t b"))
            pr_c0 = iop.tile([65, nch0, BL], BF16, tag="pr", name="pr_pre")
            nc.sync.dma_start(pr_c0, pr3[0:nch0].rearrange("t p b -> p t b"))

            # ---- load weights ----
            def cload(name):
                shape = dict(_WSPECS)[name]
                t = cp.tile(list(shape), BF16, name=f"c_{name}")
                nc.sync.dma_start(t, wsrc(name))
                return t

            woa_hh = cload("woa_hh")
            woa_obs = cload("woa_obs")
            woa_act = cload("woa_act")
            wmx_hh = cload("wmx_hh")
            wmx_ih = cload("wmx_ih")
            wsm_hh = cload("wsm_hh")
            wsm_obs = cload("wsm_obs")
            wsm_pr = cload("wsm_pr")
            ident = cload("wident")

            bias = {}
            if use_bias:
                blay = _blayout()
                for name, shape in _BSPECS:
                    sz = int(np.prod(shape))
                    tb = cp.tile(list(shape), F32, name=f"c_{name}")
                    nc.sync.dma_start(
                        tb, extras[blay[name]:blay[name] + sz].rearrange(
                            "(p a) -> p a", p=shape[0]))
                    bias[name] = tb

            # ---- init states ----
            h_oa = hp.tile([128, BL], BF16, tag="h_oa", name="h_oa_init")
            h_sm = hp.tile([128, BL], BF16, tag="h_sm", name="h_sm_init")
            h_mx = hp.tile([128, BL], BF16, tag="h_mx", name="h_mx_init")
            for h in (h_oa, h_sm, h_mx):
                nc.vector.memset(h, 0.0)

            GS = [slice(g * _GATE, (g + 1) * _GATE) for g in range(3)]

            def gru_elem(pfx, t_idx, p_r, p_z, p_nh, h_old, merged_sig=False):
                """Gate math given complete pre-activation psums. Returns h_new."""
                rz_s = tp.tile([128, 512], BF16, tag=f"{pfx}_rzs",
                               name=f"{pfx}_rzs_{t_idx}")
                if merged_sig and not use_bias:
                    nc.scalar.activation(rz_s, p_r, AFT.Sigmoid)
                else:
                    if merged_sig:
                        r_ap = p_r[:, 0] if len(p_r.shape) == 3 else p_r[:, 0:256]
                    else:
                        r_ap = p_r
                    bkw = ({"bias": bias[f"brz_{pfx}"][:, 0:1]} if use_bias else {})
                    nc.scalar.activation(rz_s[:, 0:256], r_ap, AFT.Sigmoid, **bkw)
                    bkw = ({"bias": bias[f"brz_{pfx}"][:, 1:2]} if use_bias else {})
                    nc.scalar.activation(rz_s[:, 256:512], p_z, AFT.Sigmoid, **bkw)
                v_s = tp.tile([128, BL], BF16, tag=f"{pfx}_v", name=f"{pfx}_v_{t_idx}")
                v_eng = nc.vector if pfx == "mx" else nc.gpsimd
                v_eng.tensor_mul(v_s, rz_s[:, 256:512], h_old)
                w_s = tp.tile([128, BL], BF16, tag=f"{pfx}_w", name=f"{pfx}_w_{t_idx}")
                if pfx == "mx":
                    nc.vector.tensor_scalar_sub(w_s, rz_s[:, 256:512], 1.0)
                else:
                    nc.gpsimd.tensor_scalar_sub(w_s, rz_s[:, 256:512], 1.0)
                t1 = tp.tile([128, BL], BF16, tag=f"{pfx}_t1", name=f"{pfx}_t1_{t_idx}")
                if use_bias:
                    nc.vector.scalar_tensor_tensor(t1, p_nh[:, 256:512],
                                                   bias[f"bnh_{pfx}"],
                                                   rz_s[:, 0:256],
                                                   OP.add, OP.mult)
                else:
                    nc.vector.tensor_mul(t1, p_nh[:, 256:512], rz_s[:, 0:256])
                n_s = tp.tile([128, BL], BF16, tag=f"{pfx}_n", name=f"{pfx}_n_{t_idx}")
                nc.tensor.matmul(p_nh[:, 0:256], ident, t1,
                                 start=False, stop=True, skip_group_check=True)
                bkw = ({"bias": bias[f"bni_{pfx}"]} if use_bias else {})
                nc.scalar.activation(n_s, p_nh[:, 0:256], AFT.Tanh, **bkw)
                u_s = tp.tile([128, BL], BF16, tag=f"{pfx}_u", name=f"{pfx}_u_{t_idx}")
                nc.vector.tensor_mul(u_s, w_s, n_s)
                h_new = hp.tile([128, BL], BF16, tag=f"h_{pfx}", name=f"h_{pfx}_{t_idx}")
                nc.vector.tensor_sub(h_new, v_s, u_s)
                return h_new

            CH = 8
            assert T_steps % 2 == 0
            obs_c = act_c = pr_c = None
            for t0 in range(0, T_steps, 2):
                pi = t0 // 2
                if t0 % CH == 0:
                    nch = min(CH, T_steps - t0)
                    ci = t0 // CH
                    if ci == 0:
                        obs_c, act_c, pr_c = obs_c0, act_c0, pr_c0
                    else:
                        obs_c = [iop.tile([128, nch, BL], BF16, tag=f"obs{k}",
                                          name=f"obs{k}_{ci}") for k in range(2)]
                        for k in range(2):
                            nc.sync.dma_start(
                                obs_c[k],
                                obs3[t0:t0 + nch, k * 128:(k + 1) * 128].rearrange(
                                    "t p b -> p t b"))
                        act_c = iop.tile([64, nch, BL], BF16, tag="act",
                                         name=f"act_{ci}")
                        nc.sync.dma_start(
                            act_c, act3[t0:t0 + nch].rearrange("t p b -> p t b"))
                        pr_c = iop.tile([65, nch, BL], BF16, tag="pr",
                                        name=f"pr_{ci}")
                        nc.sync.dma_start(
                            pr_c, pr3[t0:t0 + nch].rearrange("t p b -> p t b"))
                sc = t0 % CH

                # pair-batched r/z input projections at N=512 (both steps at once)
                ob0 = obs_c[0][:, sc:sc + 2]
                ob1 = obs_c[1][:, sc:sc + 2]
                pr2 = pr_c[:, sc:sc + 2]
                p_sm_pair = ppp.tile([128, 1024], F32, tag="sm_rzp",
                                     name=f"sm_rzp_{pi}")
                prs = {"sm": (p_sm_pair[:, 0:512], p_sm_pair[:, 512:1024])}
                for g in (0, 1):
                    psm = prs["sm"][g]
                    gsl = GS[g]
                    mm = nc.tensor.matmul
                    mm(psm, wsm_obs[:, 0][..., gsl], ob0, start=True, stop=False,
                       skip_group_check=True)
                    mm(psm, wsm_obs[:, 1][..., gsl], ob1, start=False, stop=False,
                       skip_group_check=True)
                    mm(psm, wsm_pr[:, gsl], pr2, start=False, stop=False,
                       skip_group_check=True)

                for s in (0, 1):
                    t = t0 + s
                    sl = slice(s * 256, (s + 1) * 256)
                    obs_s = [obs_c[0][:, sc + s], obs_c[1][:, sc + s]]

                    # ---- small lane step ----
                    p_r, p_z = prs["sm"]
                    sm_sig_in = p_sm_pair.rearrange(
                        "p (g t b) -> p g t b", g=2, t=2)[:, :, s]
                    hh = wsm_hh
                    nc.tensor.matmul(p_r[:, sl], hh[:, GS[0]], h_sm,
                                     start=False, stop=(s == 1),
                                     skip_group_check=True)
                    nc.tensor.matmul(p_z[:, sl], hh[:, GS[1]], h_sm,
                                     start=False, stop=(s == 1),
                                     skip_group_check=True)
                    p_nh = pp.tile([128, 512], F32, tag="sm_nh", name=f"sm_nh_{t}")
                    nc.tensor.matmul(p_nh[:, 256:512], hh[:, GS[2]], h_sm,
                                     start=True, stop=True)
                    xn = [(wsm_obs[:, 0][..., GS[2]], obs_s[0]),
                          (wsm_obs[:, 1][..., GS[2]], obs_s[1]),
                          (wsm_pr[:, GS[2]], pr_c[:, sc + s])]
                    for i, (w, x) in enumerate(xn):
                        nc.tensor.matmul(p_nh[:, 0:256], w, x,
                                         start=(i == 0), stop=False,
                                         skip_group_check=True)
                    h_sm = gru_elem("sm", t, sm_sig_in, p_z[:, sl], p_nh, h_sm,
                                    merged_sig=True)

                    # ---- oa lane step ----
                    p_rz_oa = pp2.tile([128, 512], F32, tag="oa_rz", name=f"oa_rz_{t}")
                    for g, psl in ((0, slice(0, 256)), (1, slice(256, 512))):
                        ihs = [(woa_obs[:, 0][..., GS[g]], obs_s[0]),
                               (woa_obs[:, 1][..., GS[g]], obs_s[1]),
                               (woa_act[:, GS[g]], act_c[:, sc + s]),
                               (woa_hh[:, GS[g]], h_oa)]
                        for i, (wt, x) in enumerate(ihs):
                            nc.tensor.matmul(p_rz_oa[:, psl], wt, x,
                                             start=(i == 0), stop=(i == 3))
                    p_nh = pp.tile([128, 512], F32, tag="oa_nh", name=f"oa_nh_{t}")
                    nc.tensor.matmul(p_nh[:, 256:512], woa_hh[:, GS[2]], h_oa,
                                     start=True, stop=True)
                    xn = [(woa_obs[:, 0][..., GS[2]], obs_s[0]),
                          (woa_obs[:, 1][..., GS[2]], obs_s[1]),
                          (woa_act[:, GS[2]], act_c[:, sc + s])]
                    for i, (w, x) in enumerate(xn):
                        nc.tensor.matmul(p_nh[:, 0:256], w, x,
                                         start=(i == 0), stop=False,
                                         skip_group_check=True)
                    h_oa = gru_elem("oa", t, p_rz_oa, p_rz_oa[:, 256:512], p_nh, h_oa,
                                    merged_sig=True)
                    # ---- mx lane step (consumes fresh h_sm) ----
                    p_rz = pp.tile([128, 512], F32, tag="mx_rz", name=f"mx_rz_{t}")
                    p_nh2 = pp.tile([128, 512], F32, tag="mx_nh", name=f"mx_nh_{t}")
                    for g, psl in ((0, slice(0, 256)), (1, slice(256, 512))):
                        nc.tensor.matmul(p_rz[:, psl], wmx_hh[:, GS[g]], h_mx,
                                         start=True, stop=False)
                        nc.tensor.matmul(p_rz[:, psl], wmx_ih[:, GS[g]], h_sm,
                                         start=False, stop=True)
                    nc.tensor.matmul(p_nh2[:, 256:512], wmx_hh[:, GS[2]], h_mx,
                                     start=True, stop=True)
                    nc.tensor.matmul(p_nh2[:, 0:256], wmx_ih[:, GS[2]], h_sm,
                                     start=True, stop=False,
                                     skip_group_check=True)
                    h_mx = gru_elem("mx", t, p_rz, p_rz[:, 256:512],
                                    p_nh2, h_mx, merged_sig=True)

            # ---- value MLP on last states: feat = [h_oa; h_mx] ----
            w1t = cload("w1t")
            w2t = cload("w2t")
            h1 = []
            for m in range(2):
                p = pp.tile([128, BL], F32, tag=("oa_nh", "sm_nh")[m], name=f"p_h1_{m}")
                ms = slice(m * 128, (m + 1) * 128)
                nc.tensor.matmul(p, w1t[:, 0, ms], h_oa, start=True, stop=False)
                nc.tensor.matmul(p, w1t[:, 1, ms], h_mx, start=False, stop=True)
                h = tp.tile([128, BL], BF16, tag=f"h1_{m}", name=f"h1_{m}")
                bkw = ({"bias": bias["b1"][:, m:m + 1]} if use_bias else {})
                nc.scalar.activation(h, p, AFT.Tanh, **bkw)
                h1.append(h)
            p_val = pp.tile([1, BL], F32, tag="mx_rz", name="p_val")
            nc.tensor.matmul(p_val, w2t[:, 0], h1[0], start=True, stop=False)
            nc.tensor.matmul(p_val, w2t[:, 1], h1[1], start=False, stop=True)
            out_s = tp.tile([1, BL], F32, tag="out", name="out_s")
            if use_bias:
                nc.scalar.activation(out_s, p_val, AFT.Identity,
                                     bias=bias["b2"][0:1, 0:1])
            else:
                nc.scalar.activation(out_s, p_val, AFT.Identity)
            nc.sync.dma_start(val[:], out_s)

    nc.compile()
    return nc


def _prep_weights(inp: dict) -> dict:
    f4 = np.float32
    g = lambda w, i: np.asarray(w)[i * (w.shape[0] // 3):(i + 1) * (w.shape[0] // 3), :]
    out = {}
    out["woa_hh"] = np.ascontiguousarray(np.asarray(inp["oa_whh"]).T, f4)
    wih_oa_t = np.asarray(inp["oa_wih"]).T  # [320, 384]
    out["woa_obs"] = np.ascontiguousarray(
        wih_oa_t[0:256].reshape(2, 128, 384).transpose(1, 0, 2), f4)
    out["woa_act"] = np.ascontiguousarray(wih_oa_t[256:320], f4)
    out["wmx_hh"] = np.ascontiguousarray(np.asarray(inp["mx_whh"]).T, f4)
    perm = np.concatenate([np.arange(64, 128), np.arange(0, 32), np.arange(32, 64)])
    out["wmx_ih"] = np.ascontiguousarray(np.asarray(inp["mx_wih"]).T[perm], f4)

    wsm_hh = np.zeros((128, 384), f4)
    wsm_obs = np.zeros((256, 384), f4)
    wsm_pr = np.zeros((65, 384), f4)
    for gi in range(3):
        c = _GATE * gi
        wsm_hh[0:64, c + 0:c + 64] = g(inp["oo_whh"], gi).T
        wsm_hh[64:96, c + 64:c + 96] = g(inp["pa_whh"], gi).T
        wsm_hh[96:128, c + 96:c + 128] = g(inp["rr_whh"], gi).T
        wsm_obs[:, c + 0:c + 64] = g(inp["oo_wih"], gi).T
        wsm_pr[0:64, c + 64:c + 96] = g(inp["pa_wih"], gi).T
        wsm_pr[64:65, c + 96:c + 128] = g(inp["rr_wih"], gi).T
    out["wsm_hh"] = wsm_hh
    out["wsm_obs"] = np.ascontiguousarray(
        wsm_obs.reshape(2, 128, 384).transpose(1, 0, 2), f4)
    out["wsm_pr"] = wsm_pr

    out["w1t"] = np.ascontiguousarray(
        np.asarray(inp["W1"]).T.reshape(2, 128, 256).transpose(1, 0, 2), f4)
    out["w2t"] = np.ascontiguousarray(
        np.asarray(inp["W2"]).T.reshape(2, 128, 1).transpose(1, 0, 2), f4)
    out["wident"] = np.eye(128, dtype=f4)

    def pack_small(v_oo, v_pa, v_rr):
        r = np.zeros(128, f4)
        r[0:64], r[64:96], r[96:128] = v_oo, v_pa, v_rr
        return r

    for key, pfx in (("oa", "oa"), ("mx", "mx")):
        bih, bhh = np.asarray(inp[f"{key}_bih"]), np.asarray(inp[f"{key}_bhh"])
        H = bih.shape[0] // 3
        out[f"bnh_{pfx}"] = np.ascontiguousarray(bhh[2 * H:3 * H], f4).reshape(128, 1)
        out[f"bni_{pfx}"] = np.ascontiguousarray(bih[2 * H:3 * H], f4).reshape(128, 1)
        out[f"brz_{pfx}"] = np.ascontiguousarray(
            np.stack([bih[0:H] + bhh[0:H], bih[H:2 * H] + bhh[H:2 * H]], 1), f4)
    bsm = {}
    for part in ("bih", "bhh"):
        vs = {k: np.asarray(inp[f"{k}_{part}"]) for k in ("oo", "pa", "rr")}
        bsm[part] = [pack_small(vs["oo"][64 * gi:64 * (gi + 1)],
                                vs["pa"][32 * gi:32 * (gi + 1)],
                                vs["rr"][32 * gi:32 * (gi + 1)]) for gi in range(3)]
    out["bnh_sm"] = bsm["bhh"][2].reshape(128, 1)
    out["bni_sm"] = bsm["bih"][2].reshape(128, 1)
    out["brz_sm"] = np.ascontiguousarray(
        np.stack([bsm["bih"][0] + bsm["bhh"][0], bsm["bih"][1] + bsm["bhh"][1]], 1), f4)
    # b1 on-chip layout is [p, m] with feature index = m*128 + p
    out["b1"] = np.ascontiguousarray(np.asarray(inp["b1"]).reshape(2, 128).T, f4)
    out["b2"] = np.ascontiguousarray(np.asarray(inp["b2"]), f4).reshape(1, 1)
    return out


def _has_bias(w: dict) -> bool:
    return any(np.abs(w[name]).max() > 0 for name, _ in _BSPECS)


def _prep_core_inputs(inp: dict, w: dict, c: int, T_steps: int,
                      use_bias: bool):
    import ml_dtypes
    bf = ml_dtypes.bfloat16
    lay = _layout(T_steps)
    flat = np.empty(lay["_total"], bf)
    bs = slice(c * BL, (c + 1) * BL)

    v = flat[lay["obs"]:lay["obs"] + T_steps * F * BL].reshape(T_steps, F, BL)
    np.copyto(v, np.asarray(inp["obs"])[:T_steps, bs, :].transpose(0, 2, 1))
    v = flat[lay["act"]:lay["act"] + T_steps * A * BL].reshape(T_steps, A, BL)
    np.copyto(v, np.asarray(inp["action"])[:T_steps, bs, :].transpose(0, 2, 1))
    v = flat[lay["pr"]:lay["pr"] + T_steps * (A + 1) * BL].reshape(
        T_steps, A + 1, BL)
    np.copyto(v[:, 0:A], np.asarray(inp["prev_action"])[:T_steps, bs, :].transpose(0, 2, 1))
    np.copyto(v[:, A:A + 1], np.asarray(inp["reward"])[:T_steps, bs, :].transpose(0, 2, 1))

    for name, shape in _WSPECS:
        sz = int(np.prod(shape))
        np.copyto(flat[lay[name]:lay[name] + sz].reshape(shape), w[name])

    m = {"flat": flat}
    if use_bias:
        blay = _blayout()
        extras = np.empty(blay["_total"], np.float32)
        for name, shape in _BSPECS:
            sz = int(np.prod(shape))
            np.copyto(extras[blay[name]:blay[name] + sz].reshape(shape),
                      w[name])
        m["extras"] = extras
    return m


_RUN_KW = {}
_CACHE = {}


def _get_nc(T_steps: int, use_bias: bool):
    from concourse.bass_interp import get_hw_module
    key = (T_steps, use_bias)
    if key not in _CACHE:
        nc = _build(T_steps, use_bias)
        nc.m = get_hw_module(nc.m)
        _CACHE[key] = nc
    return _CACHE[key]


def run(inputs: dict, T_steps: int = T, n_cores: int = NCORES):
    from concourse import bass_utils

    w = _prep_weights(inputs)
    use_bias = _has_bias(w)
    nc = _get_nc(T_steps, use_bias)
    in_maps = [_prep_core_inputs(inputs, w, c, T_steps, use_bias)
               for c in range(n_cores)]
    res = bass_utils.run_bass_kernel_spmd(
        nc, in_maps, core_ids=list(range(n_cores)), **_RUN_KW)
    vals = [res.results[c]["val"].reshape(BL) for c in range(n_cores)]
    out = np.concatenate(vals).astype(np.float32).reshape(-1, 1)
    run.last_result = res
    return out


def run_timed(inputs: dict, iters: int = 5, T_steps: int = T,
              n_cores: int = NCORES, pipeline: int = 16):
    """Wall-clocks repeated executions with device-resident inputs.
    Returns (out, seq_times, marginal_ns)."""
    import jax
    from concourse import bass2jax

    w = _prep_weights(inputs)
    use_bias = _has_bias(w)
    nc = _get_nc(T_steps, use_bias)
    in_maps = [_prep_core_inputs(inputs, w, c, T_steps, use_bias)
               for c in range(n_cores)]

    bass2jax.install_neuronx_cc_hook()
    partition_name = nc.partition_id_tensor.name if nc.partition_id_tensor else None
    in_names, out_names, out_avals, zero_outs = [], [], [], []
    import concourse.mybir as _my
    for alloc in nc.m.functions[0].allocations:
        if not isinstance(alloc, _my.MemoryLocationSet):
            continue
        name = alloc.memorylocations[0].name
        if alloc.kind == "ExternalInput":
            if name != partition_name:
                in_names.append(name)
        elif alloc.kind == "ExternalOutput":
            shape = tuple(alloc.tensor_shape)
            dtype = _my.dt.np(alloc.dtype)
            out_names.append(name)
            out_avals.append(jax.core.ShapedArray(shape, dtype))
            zero_outs.append(np.zeros(shape, dtype))
    n_params = len(in_names)
    all_in = list(in_names) + list(out_names)
    if partition_name is not None:
        all_in.append(partition_name)

    def _body(*args):
        operands = list(args)
        if partition_name is not None:
            operands.append(bass2jax.partition_id_tensor())
        outs = bass2jax._bass_exec_p.bind(
            *operands, out_avals=tuple(out_avals), in_names=tuple(all_in),
            out_names=tuple(out_names), lowering_input_output_aliases=(),
            sim_require_finite=True, sim_require_nnan=True, nc=nc)
        return tuple(outs)

    devices = jax.devices()[:n_cores]
    mesh = bass2jax.Mesh(np.asarray(devices), ("core",))
    donate = tuple(range(n_params, n_params + len(out_names)))
    sharded = jax.jit(
        bass2jax.shard_map(_body, mesh=mesh,
                           in_specs=(bass2jax.PartitionSpec("core"),) * (n_params + len(out_names)),
                           out_specs=(bass2jax.PartitionSpec("core"),) * len(out_names),
                           check_rep=False),
        donate_argnums=donate, keep_unused=True)

    concat_in = [np.concatenate([np.asarray(in_maps[c][nm]) for c in range(n_cores)], axis=0)
                 for nm in in_names]
    sh = jax.sharding.NamedSharding(mesh, bass2jax.PartitionSpec("core"))
    dev_in = [jax.device_put(x, sh) for x in concat_in]

    def zeros():
        return [jax.device_put(np.zeros((n_cores * z.shape[0], *z.shape[1:]), z.dtype), sh)
                for z in zero_outs]

    times = []
    out_arrs = None
    for _ in range(iters):
        zs = zeros()
        jax.block_until_ready(zs)
        t0 = time.time()
        out_arrs = sharded(*dev_in, *zs)
        jax.block_until_ready(out_arrs)
        times.append(time.time() - t0)

    marginal_ns = None
    if pipeline:
        tot = {}
        for N in (4, pipeline):
            best = 1e9
            for _ in range(2):
                zsets = [zeros() for _ in range(N)]
                jax.block_until_ready(zsets)
                t0 = time.time()
                outs = [sharded(*dev_in, *zs) for zs in zsets]
                jax.block_until_ready(outs)
                best = min(best, time.time() - t0)
            tot[N] = best
        marginal_ns = (tot[pipeline] - tot[4]) / (pipeline - 4) * 1e9

    res = {name: np.asarray(out_arrs[i]).reshape(n_cores, *out_avals[i].shape)
           for i, name in enumerate(out_names)}
    vals = [res["val"][c].reshape(BL) for c in range(n_cores)]
    out = np.concatenate(vals).astype(np.float32).reshape(-1, 1)
    return out, times, marginal_ns


def kernel(**inputs) -> np.ndarray:
    return run(inputs)


# revision 3
# speedup vs baseline: 1.0669x; 1.0669x over previous
"""Trainium2 Bass kernel for nn_Agent (5-GRU actor-critic encoder + value MLP).

v4 = the tuned baseline compute graph + ONE packed input tensor.

The per-dispatch runtime overhead through the PJRT path is ~34us per bound
input tensor; the previous 25-input layout spent more wall time on buffer
binding than on the kernel itself. All inputs (pre-transposed activations +
pre-arranged weights) are packed host-side into a single flat bf16 tensor
with fixed offsets; a second small fp32 tensor exists only in the (unused in
practice) nonzero-bias fallback.

Compute structure (unchanged from the tuned baseline): data-parallel over
batch (256 per core), feature-on-partitions / batch-on-free SBUF layout,
the three small GRUs packed block-diagonally into one 128-partition lane,
pair-batched small-lane r/z input projections (N=512), double-buffered
per-step oa r/z PSUM tiles, gate math split across DVE/GpSimd with the
xn + r*hn add accumulated on the TensorEngine via an identity matmul.
Zero-bias fast path: tanh reads PSUM without bias and t1 is a plain
tensor-tensor multiply.
"""

import os
import sys
import time

import numpy as np

for _p in ("/opt/trn_rl_repo", "/root/.axon_site/_ro/trn_rl_repo"):
    if _p not in sys.path and os.path.isdir(_p):
        sys.path.insert(0, _p)

import concourse.bass as bass  # noqa: E402
import concourse.mybir as mybir  # noqa: E402
import concourse.tile as tile  # noqa: E402
from concourse import bacc  # noqa: E402

F32 = mybir.dt.float32
BF16 = mybir.dt.bfloat16
AFT = mybir.ActivationFunctionType
OP = mybir.AluOpType

T, B, F, A = 128, 2048, 256, 64
NCORES = 8
BL = B // NCORES  # 256 batch per core

_GATE = 128

_WSPECS = [
    ("woa_hh", (128, 384)),
    ("woa_obs", (128, 2, 384)),
    ("woa_act", (64, 384)),
    ("wmx_hh", (128, 384)),
    ("wmx_ih", (128, 384)),
    ("wsm_hh", (128, 384)),
    ("wsm_obs", (128, 2, 384)),
    ("wsm_pr", (65, 384)),
    ("w1t", (128, 2, 256)),
    ("w2t", (128, 2, 1)),
    ("wident", (128, 128)),
]

_BSPECS = [
    ("bnh_oa", (128, 1)), ("bnh_sm", (128, 1)), ("bnh_mx", (128, 1)),
    ("bni_oa", (128, 1)), ("bni_sm", (128, 1)), ("bni_mx", (128, 1)),
    ("brz_oa", (128, 2)), ("brz_sm", (128, 2)), ("brz_mx", (128, 2)),
    ("b1", (128, 2)), ("b2", (1, 1)),
]


def _layout(T_steps: int):
    off = {}
    o = 0
    off["obs"] = o
    o += T_steps * F * BL
    off["act"] = o
    o += T_steps * A * BL
    off["pr"] = o
    o += T_steps * (A + 1) * BL
    for name, shape in _WSPECS:
        off[name] = o
        o += int(np.prod(shape))
    off["_total"] = o
    return off


def _blayout():
    off = {}
    o = 0
    for name, shape in _BSPECS:
        off[name] = o
        o += int(np.prod(shape))
    off["_total"] = o
    return off


def _build(T_steps: int, use_bias: bool):
    nc = bacc.Bacc("TRN2", target_bir_lowering=False, debug=False,
                   num_devices=1)
    lay = _layout(T_steps)
    flat = nc.dram_tensor("flat", [lay["_total"]], BF16, kind="ExternalInput")
    if use_bias:
        blay = _blayout()
        extras = nc.dram_tensor("extras", [blay["_total"]], F32,
                                kind="ExternalInput")
    val = nc.dram_tensor("val", [1, BL], F32, kind="ExternalOutput")

    obs3 = flat[lay["obs"]:lay["obs"] + T_steps * F * BL].rearrange(
        "(t p b) -> t p b", p=F, b=BL)
    act3 = flat[lay["act"]:lay["act"] + T_steps * A * BL].rearrange(
        "(t p b) -> t p b", p=A, b=BL)
    pr3 = flat[lay["pr"]:lay["pr"] + T_steps * (A + 1) * BL].rearrange(
        "(t p b) -> t p b", p=A + 1, b=BL)

    def wsrc(name):
        shape = dict(_WSPECS)[name]
        sz = int(np.prod(shape))
        ap = flat[lay[name]:lay[name] + sz]
        if len(shape) == 2:
            return ap.rearrange("(p m) -> p m", p=shape[0])
        return ap.rearrange("(p a m) -> p a m", p=shape[0], a=shape[1])

    with tile.TileContext(nc) as tc:
        with (
            tc.tile_pool(name="const", bufs=1) as cp,
            tc.tile_pool(name="io", bufs=4) as iop,
            tc.tile_pool(name="psum_pair", bufs=1, space="PSUM") as ppp,
            tc.tile_pool(name="psum", bufs=1, space="PSUM") as pp,
            tc.tile_pool(name="psum_nh2", bufs=2, space="PSUM") as pp2,
            tc.tile_pool(name="tmp", bufs=3) as tp,
            tc.tile_pool(name="state", bufs=4) as hp,
        ):
            # ---- prefetch first input chunk before weight loads ----
            nch0 = min(8, T_steps)
            obs_c0 = [iop.tile([128, nch0, BL], BF16, tag=f"obs{k}",
                               name=f"obs{k}_pre") for k in range(2)]
            for k in range(2):
                nc.sync.dma_start(
                    obs_c0[k],
                    obs3[0:nch0, k * 128:(k + 1) * 128].rearrange(
                        "t p b -> p t b"))
            act_c0 = iop.tile([64, nch0, BL], BF16, tag="act", name="act_pre")
            nc.sync.dma_start(act_c0, act3[0:nch0].rearrange("t p b -> p t b"))
            pr_c0 = iop.tile([65, nch0, BL], BF16, tag="pr", name="pr_pre")
            nc.sync.dma_start(pr_c0, pr3[0:nch0].rearrange("t p b -> p t b"))

            # ---- load weights ----
            def cload(name):
                shape = dict(_WSPECS)[name]
                t = cp.tile(list(shape), BF16, name=f"c_{name}")
                nc.sync.dma_start(t, wsrc(name))
                return t

            woa_hh = cload("woa_hh")
            woa_obs = cload("woa_obs")
            woa_act = cload("woa_act")
            wmx_hh = cload("wmx_hh")
            wmx_ih = cload("wmx_ih")
            wsm_hh = cload("wsm_hh")
            wsm_obs = cload("wsm_obs")
            wsm_pr = cload("wsm_pr")
            ident = cload("wident")

            bias = {}
            if use_bias:
                blay = _blayout()
                for name, shape in _BSPECS:
                    sz = int(np.prod(shape))
                    tb = cp.tile(list(shape), F32, name=f"c_{name}")
                    nc.sync.dma_start(
                        tb, extras[blay[name]:blay[name] + sz].rearrange(
                            "(p a) -> p a", p=shape[0]))
                    bias[name] = tb

            # ---- init states ----
            h_oa = hp.tile([128, BL], BF16, tag="h_oa", name="h_oa_init")
            h_sm = hp.tile([128, BL], BF16, tag="h_sm", name="h_sm_init")
            h_mx = hp.tile([128, BL], BF16, tag="h_mx", name="h_mx_init")
            for h in (h_oa, h_sm, h_mx):
                nc.vector.memset(h, 0.0)

            GS = [slice(g * _GATE, (g + 1) * _GATE) for g in range(3)]

            def gru_elem(pfx, t_idx, p_r, p_z, p_nh, h_old, merged_sig=False):
                """Gate math given complete pre-activation psums. Returns h_new."""
                rz_s = tp.tile([128, 512], BF16, tag=f"{pfx}_rzs",
                               name=f"{pfx}_rzs_{t_idx}")
                if merged_sig and not use_bias:
                    nc.scalar.activation(rz_s, p_r, AFT.Sigmoid)
                else:
                    if merged_sig:
                        r_ap = p_r[:, 0] if len(p_r.shape) == 3 else p_r[:, 0:256]
                    else:
                        r_ap = p_r
                    bkw = ({"bias": bias[f"brz_{pfx}"][:, 0:1]} if use_bias else {})
                    nc.scalar.activation(rz_s[:, 0:256], r_ap, AFT.Sigmoid, **bkw)
                    bkw = ({"bias": bias[f"brz_{pfx}"][:, 1:2]} if use_bias else {})
                    nc.scalar.activation(rz_s[:, 256:512], p_z, AFT.Sigmoid, **bkw)
                v_s = tp.tile([128, BL], BF16, tag=f"{pfx}_v", name=f"{pfx}_v_{t_idx}")
                v_eng = nc.vector if pfx == "mx" else nc.gpsimd
                v_eng.tensor_mul(v_s, rz_s[:, 256:512], h_old)
                w_s = tp.tile([128, BL], BF16, tag=f"{pfx}_w", name=f"{pfx}_w_{t_idx}")
                if pfx == "mx":
                    nc.vector.tensor_scalar_sub(w_s, rz_s[:, 256:512], 1.0)
                else:
                    nc.gpsimd.tensor_scalar_sub(w_s, rz_s[:, 256:512], 1.0)
                t1 = tp.tile([128, BL], BF16, tag=f"{pfx}_t1", name=f"{pfx}_t1_{t_idx}")
                if use_bias:
                    nc.vector.scalar_tensor_tensor(t1, p_nh[:, 256:512],
                                                   bias[f"bnh_{pfx}"],
                                                   rz_s[:, 0:256],
                                                   OP.add, OP.mult)
                else:
                    nc.vector.tensor_mul(t1, p_nh[:, 256:512], rz_s[:, 0:256])
                n_s = tp.tile([128, BL], BF16, tag=f"{pfx}_n", name=f"{pfx}_n_{t_idx}")
                nc.tensor.matmul(p_nh[:, 0:256], ident, t1,
                                 start=False, stop=True, skip_group_check=True)
                bkw = ({"bias": bias[f"bni_{pfx}"]} if use_bias else {})
                nc.scalar.activation(n_s, p_nh[:, 0:256], AFT.Tanh, **bkw)
                u_s = tp.tile([128, BL], BF16, tag=f"{pfx}_u", name=f"{pfx}_u_{t_idx}")
                nc.vector.tensor_mul(u_s, w_s, n_s)
                h_new = hp.tile([128, BL], BF16, tag=f"h_{pfx}", name=f"h_{pfx}_{t_idx}")
                nc.vector.tensor_sub(h_new, v_s, u_s)
                return h_new

            CH = 8
            assert T_steps % 2 == 0
            chunks = {}  # ci -> (obs_c, act_c, pr_c)
            heads = {}   # pi -> (p_sm_pair, prs)

            def emit_pair_head(t0):
                """Chunk DMA (if due) + pair-batched sm r/z x-projections."""
                pi = t0 // 2
                ci = t0 // CH
                if t0 % CH == 0 and ci not in chunks:
                    nch = min(CH, T_steps - t0)
                    if ci == 0:
                        chunks[ci] = (obs_c0, act_c0, pr_c0)
                    else:
                        obs_n = [iop.tile([128, nch, BL], BF16, tag=f"obs{k}",
                                          name=f"obs{k}_{ci}") for k in range(2)]
                        for k in range(2):
                            nc.sync.dma_start(
                                obs_n[k],
                                obs3[t0:t0 + nch, k * 128:(k + 1) * 128].rearrange(
                                    "t p b -> p t b"))
                        act_n = iop.tile([64, nch, BL], BF16, tag="act",
                                         name=f"act_{ci}")
                        nc.sync.dma_start(
                            act_n, act3[t0:t0 + nch].rearrange("t p b -> p t b"))
                        pr_n = iop.tile([65, nch, BL], BF16, tag="pr",
                                        name=f"pr_{ci}")
                        nc.sync.dma_start(
                            pr_n, pr3[t0:t0 + nch].rearrange("t p b -> p t b"))
                        chunks[ci] = (obs_n, act_n, pr_n)
                obs_h, act_h, pr_h = chunks[ci]
                sc_h = t0 % CH
                ob0 = obs_h[0][:, sc_h:sc_h + 2]
                ob1 = obs_h[1][:, sc_h:sc_h + 2]
                pr2 = pr_h[:, sc_h:sc_h + 2]
                p_sm_pair = ppp.tile([128, 1024], F32, tag="sm_rzp",
                                     name=f"sm_rzp_{pi}")
                prs = (p_sm_pair[:, 0:512], p_sm_pair[:, 512:1024])
                for g in (0, 1):
                    psm = prs[g]
                    gsl = GS[g]
                    mm = nc.tensor.matmul
                    mm(psm, wsm_obs[:, 0][..., gsl], ob0, start=True, stop=False,
                       skip_group_check=True)
                    mm(psm, wsm_obs[:, 1][..., gsl], ob1, start=False, stop=False,
                       skip_group_check=True)
                    mm(psm, wsm_pr[:, gsl], pr2, start=False, stop=False,
                       skip_group_check=True)
                heads[pi] = (p_sm_pair, prs)

            emit_pair_head(0)
            for t0 in range(0, T_steps, 2):
                pi = t0 // 2
                ci = t0 // CH
                obs_c, act_c, pr_c = chunks[ci]
                p_sm_pair, prs_pair = heads.pop(pi)
                prs = {"sm": prs_pair}
                sc = t0 % CH

                for s in (0, 1):
                    t = t0 + s
                    sl = slice(s * 256, (s + 1) * 256)
                    obs_s = [obs_c[0][:, sc + s], obs_c[1][:, sc + s]]

                    # ---- small lane step ----
                    p_r, p_z = prs["sm"]
                    sm_sig_in = p_sm_pair.rearrange(
                        "p (g t b) -> p g t b", g=2, t=2)[:, :, s]
                    hh = wsm_hh
                    nc.tensor.matmul(p_r[:, sl], hh[:, GS[0]], h_sm,
                                     start=False, stop=(s == 1),
                                     skip_group_check=True)
                    nc.tensor.matmul(p_z[:, sl], hh[:, GS[1]], h_sm,
                                     start=False, stop=(s == 1),
                                     skip_group_check=True)
                    p_nh = pp.tile([128, 512], F32, tag="sm_nh", name=f"sm_nh_{t}")
                    nc.tensor.matmul(p_nh[:, 256:512], hh[:, GS[2]], h_sm,
                                     start=True, stop=True)
                    xn = [(wsm_obs[:, 0][..., GS[2]], obs_s[0]),
                          (wsm_obs[:, 1][..., GS[2]], obs_s[1]),
                          (wsm_pr[:, GS[2]], pr_c[:, sc + s])]
                    for i, (w, x) in enumerate(xn):
                        nc.tensor.matmul(p_nh[:, 0:256], w, x,
                                         start=(i == 0), stop=False,
                                         skip_group_check=True)
                    h_sm_prev = h_sm

                    # ---- oa lane matmuls (h_oa + x only) ----
                    p_rz_oa = pp2.tile([128, 512], F32, tag="oa_rz", name=f"oa_rz_{t}")
                    for g, psl in ((0, slice(0, 256)), (1, slice(256, 512))):
                        ihs = [(woa_obs[:, 0][..., GS[g]], obs_s[0]),
                               (woa_obs[:, 1][..., GS[g]], obs_s[1]),
                               (woa_act[:, GS[g]], act_c[:, sc + s]),
                               (woa_hh[:, GS[g]], h_oa)]
                        for i, (wt, x) in enumerate(ihs):
                            nc.tensor.matmul(p_rz_oa[:, psl], wt, x,
                                             start=(i == 0), stop=(i == 3))
                    p_nh_oa = pp.tile([128, 512], F32, tag="oa_nh", name=f"oa_nh_{t}")
                    nc.tensor.matmul(p_nh_oa[:, 256:512], woa_hh[:, GS[2]], h_oa,
                                     start=True, stop=True)
                    xn = [(woa_obs[:, 0][..., GS[2]], obs_s[0]),
                          (woa_obs[:, 1][..., GS[2]], obs_s[1]),
                          (woa_act[:, GS[2]], act_c[:, sc + s])]
                    for i, (w, x) in enumerate(xn):
                        nc.tensor.matmul(p_nh_oa[:, 0:256], w, x,
                                         start=(i == 0), stop=False,
                                         skip_group_check=True)

                    # ---- mx matmuls that need only h_mx ----
                    p_rz = pp.tile([128, 512], F32, tag="mx_rz", name=f"mx_rz_{t}")
                    for g, psl in ((0, slice(0, 256)), (1, slice(256, 512))):
                        nc.tensor.matmul(p_rz[:, psl], wmx_hh[:, GS[g]], h_mx,
                                         start=(g == 0), stop=False,
                                         skip_group_check=True)
                    p_nh2 = pp.tile([128, 512], F32, tag="mx_nh", name=f"mx_nh_{t}")
                    nc.tensor.matmul(p_nh2[:, 256:512], wmx_hh[:, GS[2]], h_mx,
                                     start=True, stop=True)

                    # ---- gate chains ----
                    h_sm = gru_elem("sm", t, sm_sig_in, p_z[:, sl], p_nh,
                                    h_sm, merged_sig=True)
                    h_oa = gru_elem("oa", t, p_rz_oa, p_rz_oa[:, 256:512],
                                    p_nh_oa, h_oa, merged_sig=True)

                    # ---- mx matmuls on fresh h_sm, then mx gates ----
                    for g, psl in ((0, slice(0, 256)), (1, slice(256, 512))):
                        nc.tensor.matmul(p_rz[:, psl], wmx_ih[:, GS[g]], h_sm,
                                         start=False, stop=(g == 1),
                                         skip_group_check=True)
                    nc.tensor.matmul(p_nh2[:, 0:256], wmx_ih[:, GS[2]], h_sm,
                                     start=True, stop=False,
                                     skip_group_check=True)
                    if s == 1 and t0 + 2 < T_steps:
                        emit_pair_head(t0 + 2)
                    h_mx = gru_elem("mx", t, p_rz, p_rz[:, 256:512],
                                    p_nh2, h_mx, merged_sig=True)

            # ---- value MLP on last states: feat = [h_oa; h_mx] ----
            w1t = cload("w1t")
            w2t = cload("w2t")
            h1 = []
            for m in range(2):
                p = pp.tile([128, BL], F32, tag=("oa_nh", "sm_nh")[m], name=f"p_h1_{m}")
                ms = slice(m * 128, (m + 1) * 128)
                nc.tensor.matmul(p, w1t[:, 0, ms], h_oa, start=True, stop=False)
                nc.tensor.matmul(p, w1t[:, 1, ms], h_mx, start=False, stop=True)
                h = tp.tile([128, BL], BF16, tag=f"h1_{m}", name=f"h1_{m}")
                bkw = ({"bias": bias["b1"][:, m:m + 1]} if use_bias else {})
                nc.scalar.activation(h, p, AFT.Tanh, **bkw)
                h1.append(h)
            p_val = pp.tile([1, BL], F32, tag="mx_rz", name="p_val")
            nc.tensor.matmul(p_val, w2t[:, 0], h1[0], start=True, stop=False)
            nc.tensor.matmul(p_val, w2t[:, 1], h1[1], start=False, stop=True)
            out_s = tp.tile([1, BL], F32, tag="out", name="out_s")
            if use_bias:
                nc.scalar.activation(out_s, p_val, AFT.Identity,
                                     bias=bias["b2"][0:1, 0:1])
            else:
                nc.scalar.activation(out_s, p_val, AFT.Identity)
            nc.sync.dma_start(val[:], out_s)

    nc.compile()
    return nc


def _prep_weights(inp: dict) -> dict:
    f4 = np.float32
    g = lambda w, i: np.asarray(w)[i * (w.shape[0] // 3):(i + 1) * (w.shape[0] // 3), :]
    out = {}
    out["woa_hh"] = np.ascontiguousarray(np.asarray(inp["oa_whh"]).T, f4)
    wih_oa_t = np.asarray(inp["oa_wih"]).T  # [320, 384]
    out["woa_obs"] = np.ascontiguousarray(
        wih_oa_t[0:256].reshape(2, 128, 384).transpose(1, 0, 2), f4)
    out["woa_act"] = np.ascontiguousarray(wih_oa_t[256:320], f4)
    out["wmx_hh"] = np.ascontiguousarray(np.asarray(inp["mx_whh"]).T, f4)
    perm = np.concatenate([np.arange(64, 128), np.arange(0, 32), np.arange(32, 64)])
    out["wmx_ih"] = np.ascontiguousarray(np.asarray(inp["mx_wih"]).T[perm], f4)

    wsm_hh = np.zeros((128, 384), f4)
    wsm_obs = np.zeros((256, 384), f4)
    wsm_pr = np.zeros((65, 384), f4)
    for gi in range(3):
        c = _GATE * gi
        wsm_hh[0:64, c + 0:c + 64] = g(inp["oo_whh"], gi).T
        wsm_hh[64:96, c + 64:c + 96] = g(inp["pa_whh"], gi).T
        wsm_hh[96:128, c + 96:c + 128] = g(inp["rr_whh"], gi).T
        wsm_obs[:, c + 0:c + 64] = g(inp["oo_wih"], gi).T
        wsm_pr[0:64, c + 64:c + 96] = g(inp["pa_wih"], gi).T
        wsm_pr[64:65, c + 96:c + 128] = g(inp["rr_wih"], gi).T
    out["wsm_hh"] = wsm_hh
    out["wsm_obs"] = np.ascontiguousarray(
        wsm_obs.reshape(2, 128, 384).transpose(1, 0, 2), f4)
    out["wsm_pr"] = wsm_pr

    out["w1t"] = np.ascontiguousarray(
        np.asarray(inp["W1"]).T.reshape(2, 128, 256).transpose(1, 0, 2), f4)
    out["w2t"] = np.ascontiguousarray(
        np.asarray(inp["W2"]).T.reshape(2, 128, 1).transpose(1, 0, 2), f4)
    out["wident"] = np.eye(128, dtype=f4)

    def pack_small(v_oo, v_pa, v_rr):
        r = np.zeros(128, f4)
        r[0:64], r[64:96], r[96:128] = v_oo, v_pa, v_rr
        return r

    for key, pfx in (("oa", "oa"), ("mx", "mx")):
        bih, bhh = np.asarray(inp[f"{key}_bih"]), np.asarray(inp[f"{key}_bhh"])
        H = bih.shape[0] // 3
        out[f"bnh_{pfx}"] = np.ascontiguousarray(bhh[2 * H:3 * H], f4).reshape(128, 1)
        out[f"bni_{pfx}"] = np.ascontiguousarray(bih[2 * H:3 * H], f4).reshape(128, 1)
        out[f"brz_{pfx}"] = np.ascontiguousarray(
            np.stack([bih[0:H] + bhh[0:H], bih[H:2 * H] + bhh[H:2 * H]], 1), f4)
    bsm = {}
    for part in ("bih", "bhh"):
        vs = {k: np.asarray(inp[f"{k}_{part}"]) for k in ("oo", "pa", "rr")}
        bsm[part] = [pack_small(vs["oo"][64 * gi:64 * (gi + 1)],
                                vs["pa"][32 * gi:32 * (gi + 1)],
                                vs["rr"][32 * gi:32 * (gi + 1)]) for gi in range(3)]
    out["bnh_sm"] = bsm["bhh"][2].reshape(128, 1)
    out["bni_sm"] = bsm["bih"][2].reshape(128, 1)
    out["brz_sm"] = np.ascontiguousarray(
        np.stack([bsm["bih"][0] + bsm["bhh"][0], bsm["bih"][1] + bsm["bhh"][1]], 1), f4)
    # b1 on-chip layout is [p, m] with feature index = m*128 + p
    out["b1"] = np.ascontiguousarray(np.asarray(inp["b1"]).reshape(2, 128).T, f4)
    out["b2"] = np.ascontiguousarray(np.asarray(inp["b2"]), f4).reshape(1, 1)
    return out


def _has_bias(w: dict) -> bool:
    return any(np.abs(w[name]).max() > 0 for name, _ in _BSPECS)


def _prep_core_inputs(inp: dict, w: dict, c: int, T_steps: int,
                      use_bias: bool):
    import ml_dtypes
    bf = ml_dtypes.bfloat16
    lay = _layout(T_steps)
    flat = np.empty(lay["_total"], bf)
    bs = slice(c * BL, (c + 1) * BL)

    v = flat[lay["obs"]:lay["obs"] + T_steps * F * BL].reshape(T_steps, F, BL)
    np.copyto(v, np.asarray(inp["obs"])[:T_steps, bs, :].transpose(0, 2, 1))
    v = flat[lay["act"]:lay["act"] + T_steps * A * BL].reshape(T_steps, A, BL)
    np.copyto(v, np.asarray(inp["action"])[:T_steps, bs, :].transpose(0, 2, 1))
    v = flat[lay["pr"]:lay["pr"] + T_steps * (A + 1) * BL].reshape(
        T_steps, A + 1, BL)
    np.copyto(v[:, 0:A], np.asarray(inp["prev_action"])[:T_steps, bs, :].transpose(0, 2, 1))
    np.copyto(v[:, A:A + 1], np.asarray(inp["reward"])[:T_steps, bs, :].transpose(0, 2, 1))

    for name, shape in _WSPECS:
        sz = int(np.prod(shape))
        np.copyto(flat[lay[name]:lay[name] + sz].reshape(shape), w[name])

    m = {"flat": flat}
    if use_bias:
        blay = _blayout()
        extras = np.empty(blay["_total"], np.float32)
        for name, shape in _BSPECS:
            sz = int(np.prod(shape))
            np.copyto(extras[blay[name]:blay[name] + sz].reshape(shape),
                      w[name])
        m["extras"] = extras
    return m


_RUN_KW = {}
_CACHE = {}


def _get_nc(T_steps: int, use_bias: bool):
    from concourse.bass_interp import get_hw_module
    key = (T_steps, use_bias)
    if key not in _CACHE:
        nc = _build(T_steps, use_bias)
        nc.m = get_hw_module(nc.m)
        _CACHE[key] = nc
    return _CACHE[key]


def run(inputs: dict, T_steps: int = T, n_cores: int = NCORES):
    from concourse import bass_utils

    w = _prep_weights(inputs)
    use_bias = _has_bias(w)
    nc = _get_nc(T_steps, use_bias)
    in_maps = [_prep_core_inputs(inputs, w, c, T_steps, use_bias)
               for c in range(n_cores)]
    res = bass_utils.run_bass_kernel_spmd(
        nc, in_maps, core_ids=list(range(n_cores)), **_RUN_KW)
    vals = [res.results[c]["val"].reshape(BL) for c in range(n_cores)]
    out = np.concatenate(vals).astype(np.float32).reshape(-1, 1)
    run.last_result = res
    return out


def run_timed(inputs: dict, iters: int = 5, T_steps: int = T,
              n_cores: int = NCORES, pipeline: int = 16):
    """Wall-clocks repeated executions with device-resident inputs.
    Returns (out, seq_times, marginal_ns)."""
    import jax
    from concourse import bass2jax

    w = _prep_weights(inputs)
    use_bias = _has_bias(w)
    nc = _get_nc(T_steps, use_bias)
    in_maps = [_prep_core_inputs(inputs, w, c, T_steps, use_bias)
               for c in range(n_cores)]

    bass2jax.install_neuronx_cc_hook()
    partition_name = nc.partition_id_tensor.name if nc.partition_id_tensor else None
    in_names, out_names, out_avals, zero_outs = [], [], [], []
    import concourse.mybir as _my
    for alloc in nc.m.functions[0].allocations:
        if not isinstance(alloc, _my.MemoryLocationSet):
            continue
        name = alloc.memorylocations[0].name
        if alloc.kind == "ExternalInput":
            if name != partition_name:
                in_names.append(name)
        elif alloc.kind == "ExternalOutput":
            shape = tuple(alloc.tensor_shape)
            dtype = _my.dt.np(alloc.dtype)
            out_names.append(name)
            out_avals.append(jax.core.ShapedArray(shape, dtype))
            zero_outs.append(np.zeros(shape, dtype))
    n_params = len(in_names)
    all_in = list(in_names) + list(out_names)
    if partition_name is not None:
        all_in.append(partition_name)

    def _body(*args):
        operands = list(args)
        if partition_name is not None:
            operands.append(bass2jax.partition_id_tensor())
        outs = bass2jax._bass_exec_p.bind(
            *operands, out_avals=tuple(out_avals), in_names=tuple(all_in),
            out_names=tuple(out_names), lowering_input_output_aliases=(),
            sim_require_finite=True, sim_require_nnan=True, nc=nc)
        return tuple(outs)

    devices = jax.devices()[:n_cores]
    mesh = bass2jax.Mesh(np.asarray(devices), ("core",))
    donate = tuple(range(n_params, n_params + len(out_names)))
    sharded = jax.jit(
        bass2jax.shard_map(_body, mesh=mesh,
                           in_specs=(bass2jax.PartitionSpec("core"),) * (n_params + len(out_names)),
                           out_specs=(bass2jax.PartitionSpec("core"),) * len(out_names),
                           check_rep=False),
        donate_argnums=donate, keep_unused=True)

    concat_in = [np.concatenate([np.asarray(in_maps[c][nm]) for c in range(n_cores)], axis=0)
                 for nm in in_names]
    sh = jax.sharding.NamedSharding(mesh, bass2jax.PartitionSpec("core"))
    dev_in = [jax.device_put(x, sh) for x in concat_in]

    def zeros():
        return [jax.device_put(np.zeros((n_cores * z.shape[0], *z.shape[1:]), z.dtype), sh)
                for z in zero_outs]

    times = []
    out_arrs = None
    for _ in range(iters):
        zs = zeros()
        jax.block_until_ready(zs)
        t0 = time.time()
        out_arrs = sharded(*dev_in, *zs)
        jax.block_until_ready(out_arrs)
        times.append(time.time() - t0)

    marginal_ns = None
    if pipeline:
        tot = {}
        for N in (4, pipeline):
            best = 1e9
            for _ in range(2):
                zsets = [zeros() for _ in range(N)]
                jax.block_until_ready(zsets)
                t0 = time.time()
                outs = [sharded(*dev_in, *zs) for zs in zsets]
                jax.block_until_ready(outs)
                best = min(best, time.time() - t0)
            tot[N] = best
        marginal_ns = (tot[pipeline] - tot[4]) / (pipeline - 4) * 1e9

    res = {name: np.asarray(out_arrs[i]).reshape(n_cores, *out_avals[i].shape)
           for i, name in enumerate(out_names)}
    vals = [res["val"][c].reshape(BL) for c in range(n_cores)]
    out = np.concatenate(vals).astype(np.float32).reshape(-1, 1)
    return out, times, marginal_ns


def kernel(**inputs) -> np.ndarray:
    return run(inputs)


# revision 5
# speedup vs baseline: 1.2734x; 1.1936x over previous
"""Trainium2 Bass kernel for nn_Agent (5-GRU actor-critic encoder + value MLP).

v4 = the tuned baseline compute graph + ONE packed input tensor.

The per-dispatch runtime overhead through the PJRT path is ~34us per bound
input tensor; the previous 25-input layout spent more wall time on buffer
binding than on the kernel itself. All inputs (pre-transposed activations +
pre-arranged weights) are packed host-side into a single flat bf16 tensor
with fixed offsets; a second small fp32 tensor exists only in the (unused in
practice) nonzero-bias fallback.

Compute structure (unchanged from the tuned baseline): data-parallel over
batch (256 per core), feature-on-partitions / batch-on-free SBUF layout,
the three small GRUs packed block-diagonally into one 128-partition lane,
pair-batched small-lane r/z input projections (N=512), double-buffered
per-step oa r/z PSUM tiles, gate math split across DVE/GpSimd with the
xn + r*hn add accumulated on the TensorEngine via an identity matmul.
Zero-bias fast path: tanh reads PSUM without bias and t1 is a plain
tensor-tensor multiply.
"""

import os
import sys
import time

import numpy as np

for _p in ("/opt/trn_rl_repo", "/root/.axon_site/_ro/trn_rl_repo"):
    if _p not in sys.path and os.path.isdir(_p):
        sys.path.insert(0, _p)

import concourse.bass as bass  # noqa: E402
import concourse.mybir as mybir  # noqa: E402
import concourse.tile as tile  # noqa: E402
from concourse import bacc  # noqa: E402

F32 = mybir.dt.float32
BF16 = mybir.dt.bfloat16
AFT = mybir.ActivationFunctionType
OP = mybir.AluOpType

T, B, F, A = 128, 2048, 256, 64
NCORES = 8
BL = B // NCORES  # 256 batch per core

_GATE = 128

_WSPECS = [
    ("woa_hh", (128, 384)),
    ("woa_obs", (128, 2, 384)),
    ("woa_act", (64, 384)),
    ("wmx_hh", (128, 384)),
    ("wmx_ih", (128, 384)),
    ("wsm_hh", (128, 384)),
    ("wsm_obs", (128, 2, 384)),
    ("wsm_pr", (65, 384)),
    ("w1t", (128, 2, 256)),
    ("w2t", (128, 2, 1)),
    ("wident", (128, 128)),
]

_BSPECS = [
    ("bnh_oa", (128, 1)), ("bnh_sm", (128, 1)), ("bnh_mx", (128, 1)),
    ("bni_oa", (128, 1)), ("bni_sm", (128, 1)), ("bni_mx", (128, 1)),
    ("brz_oa", (128, 2)), ("brz_sm", (128, 2)), ("brz_mx", (128, 2)),
    ("b1", (128, 2)), ("b2", (1, 1)),
]


_CH = 8


def _layout(T_steps: int):
    off = {}
    o = 0
    off["obs"] = o
    o += T_steps * F * BL
    off["act"] = o
    o += T_steps * A * BL
    off["pr"] = o
    o += T_steps * (A + 1) * BL
    for name, shape in _WSPECS:
        off[name] = o
        o += int(np.prod(shape))
    off["_total"] = o
    return off


def _blayout():
    off = {}
    o = 0
    for name, shape in _BSPECS:
        off[name] = o
        o += int(np.prod(shape))
    off["_total"] = o
    return off


def _build(T_steps: int, use_bias: bool):
    nc = bacc.Bacc("TRN2", target_bir_lowering=False, debug=False,
                   num_devices=1)
    lay = _layout(T_steps)
    flat = nc.dram_tensor("flat", [lay["_total"]], BF16, kind="ExternalInput")
    if use_bias:
        blay = _blayout()
        extras = nc.dram_tensor("extras", [blay["_total"]], F32,
                                kind="ExternalInput")
    val = nc.dram_tensor("val", [1, BL], F32, kind="ExternalOutput")

    def chunk_src(kind, t0, nch, k=0):
        # contiguous chunk blocks: obs [ci][k][128][CH][BL], act [ci][64][CH][BL],
        # pr [ci][65][CH][BL]
        if kind == "obs":
            o = lay["obs"] + (t0 * F + k * 128 * nch) * BL
            return flat[o:o + 128 * nch * BL].rearrange(
                "(p t b) -> p t b", p=128, t=nch)
        if kind == "act":
            o = lay["act"] + t0 * A * BL
            return flat[o:o + A * nch * BL].rearrange(
                "(p t b) -> p t b", p=A, t=nch)
        o = lay["pr"] + t0 * (A + 1) * BL
        return flat[o:o + (A + 1) * nch * BL].rearrange(
            "(p t b) -> p t b", p=A + 1, t=nch)

    def wsrc(name):
        shape = dict(_WSPECS)[name]
        sz = int(np.prod(shape))
        ap = flat[lay[name]:lay[name] + sz]
        if len(shape) == 2:
            return ap.rearrange("(p m) -> p m", p=shape[0])
        return ap.rearrange("(p a m) -> p a m", p=shape[0], a=shape[1])

    with tile.TileContext(nc) as tc:
        with (
            tc.tile_pool(name="const", bufs=1) as cp,
            tc.tile_pool(name="io", bufs=4) as iop,
            tc.tile_pool(name="psum_pair", bufs=1, space="PSUM") as ppp,
            tc.tile_pool(name="psum", bufs=1, space="PSUM") as pp,
            tc.tile_pool(name="psum_nh2", bufs=2, space="PSUM") as pp2,
            tc.tile_pool(name="tmp", bufs=3) as tp,
            tc.tile_pool(name="state", bufs=4) as hp,
        ):
            # ---- prefetch first input chunk before weight loads ----
            nch0 = min(_CH, T_steps)
            obs_c0 = [iop.tile([128, nch0, BL], BF16, tag=f"obs{k}",
                               name=f"obs{k}_pre") for k in range(2)]
            for k in range(2):
                nc.sync.dma_start(obs_c0[k], chunk_src("obs", 0, nch0, k))
            act_c0 = iop.tile([64, nch0, BL], BF16, tag="act", name="act_pre")
            nc.sync.dma_start(act_c0, chunk_src("act", 0, nch0))
            pr_c0 = iop.tile([65, nch0, BL], BF16, tag="pr", name="pr_pre")
            nc.sync.dma_start(pr_c0, chunk_src("pr", 0, nch0))

            # ---- load weights ----
            def cload(name):
                shape = dict(_WSPECS)[name]
                t = cp.tile(list(shape), BF16, name=f"c_{name}")
                nc.sync.dma_start(t, wsrc(name))
                return t

            woa_hh = cload("woa_hh")
            woa_obs = cload("woa_obs")
            woa_act = cload("woa_act")
            wmx_hh = cload("wmx_hh")
            wmx_ih = cload("wmx_ih")
            wsm_hh = cload("wsm_hh")
            wsm_obs = cload("wsm_obs")
            wsm_pr = cload("wsm_pr")
            ident = cload("wident")

            bias = {}
            if use_bias:
                blay = _blayout()
                for name, shape in _BSPECS:
                    sz = int(np.prod(shape))
                    tb = cp.tile(list(shape), F32, name=f"c_{name}")
                    nc.sync.dma_start(
                        tb, extras[blay[name]:blay[name] + sz].rearrange(
                            "(p a) -> p a", p=shape[0]))
                    bias[name] = tb

            # ---- init states ----
            h_oa = hp.tile([128, BL], BF16, tag="h_oa", name="h_oa_init")
            h_sm = hp.tile([128, BL], BF16, tag="h_sm", name="h_sm_init")
            h_mx = hp.tile([128, BL], BF16, tag="h_mx", name="h_mx_init")
            for h in (h_oa, h_sm, h_mx):
                nc.vector.memset(h, 0.0)

            GS = [slice(g * _GATE, (g + 1) * _GATE) for g in range(3)]

            def gru_elem(pfx, t_idx, p_r, p_z, p_nh, h_old, merged_sig=False):
                """Gate math given complete pre-activation psums. Returns h_new."""
                rz_s = tp.tile([128, 512], BF16, tag=f"{pfx}_rzs",
                               name=f"{pfx}_rzs_{t_idx}")
                if merged_sig and not use_bias:
                    nc.scalar.activation(rz_s, p_r, AFT.Sigmoid)
                else:
                    if merged_sig:
                        r_ap = p_r[:, 0] if len(p_r.shape) == 3 else p_r[:, 0:256]
                    else:
                        r_ap = p_r
                    bkw = ({"bias": bias[f"brz_{pfx}"][:, 0:1]} if use_bias else {})
                    nc.scalar.activation(rz_s[:, 0:256], r_ap, AFT.Sigmoid, **bkw)
                    bkw = ({"bias": bias[f"brz_{pfx}"][:, 1:2]} if use_bias else {})
                    nc.scalar.activation(rz_s[:, 256:512], p_z, AFT.Sigmoid, **bkw)
                v_s = tp.tile([128, BL], BF16, tag=f"{pfx}_v", name=f"{pfx}_v_{t_idx}")
                v_eng = nc.vector if pfx == "mx" else nc.gpsimd
                v_eng.tensor_mul(v_s, rz_s[:, 256:512], h_old)
                w_s = tp.tile([128, BL], BF16, tag=f"{pfx}_w", name=f"{pfx}_w_{t_idx}")
                if pfx == "mx":
                    nc.vector.tensor_scalar_sub(w_s, rz_s[:, 256:512], 1.0)
                else:
                    nc.gpsimd.tensor_scalar_sub(w_s, rz_s[:, 256:512], 1.0)
                t1 = tp.tile([128, BL], BF16, tag=f"{pfx}_t1", name=f"{pfx}_t1_{t_idx}")
                if use_bias:
                    nc.vector.scalar_tensor_tensor(t1, p_nh[:, 256:512],
                                                   bias[f"bnh_{pfx}"],
                                                   rz_s[:, 0:256],
                                                   OP.add, OP.mult)
                else:
                    nc.vector.tensor_mul(t1, p_nh[:, 256:512], rz_s[:, 0:256])
                n_s = tp.tile([128, BL], BF16, tag=f"{pfx}_n", name=f"{pfx}_n_{t_idx}")
                nc.tensor.matmul(p_nh[:, 0:256], ident, t1,
                                 start=False, stop=True, skip_group_check=True)
                bkw = ({"bias": bias[f"bni_{pfx}"]} if use_bias else {})
                nc.scalar.activation(n_s, p_nh[:, 0:256], AFT.Tanh, **bkw)
                u_s = tp.tile([128, BL], BF16, tag=f"{pfx}_u", name=f"{pfx}_u_{t_idx}")
                nc.vector.tensor_mul(u_s, w_s, n_s)
                h_new = hp.tile([128, BL], BF16, tag=f"h_{pfx}", name=f"h_{pfx}_{t_idx}")
                nc.vector.tensor_sub(h_new, v_s, u_s)
                return h_new

            CH = _CH
            assert T_steps % 2 == 0
            chunks = {}  # ci -> (obs_c, act_c, pr_c)
            heads = {}   # pi -> (p_sm_pair, prs)

            def emit_pair_head(t0):
                """Chunk DMA (if due) + pair-batched sm r/z x-projections."""
                pi = t0 // 2
                ci = t0 // CH
                if t0 % CH == 0 and ci not in chunks:
                    nch = min(CH, T_steps - t0)
                    if ci == 0:
                        chunks[ci] = (obs_c0, act_c0, pr_c0)
                    else:
                        obs_n = [iop.tile([128, nch, BL], BF16, tag=f"obs{k}",
                                          name=f"obs{k}_{ci}") for k in range(2)]
                        for k in range(2):
                            nc.sync.dma_start(obs_n[k],
                                              chunk_src("obs", t0, nch, k))
                        act_n = iop.tile([64, nch, BL], BF16, tag="act",
                                         name=f"act_{ci}")
                        nc.sync.dma_start(act_n, chunk_src("act", t0, nch))
                        pr_n = iop.tile([65, nch, BL], BF16, tag="pr",
                                        name=f"pr_{ci}")
                        nc.sync.dma_start(pr_n, chunk_src("pr", t0, nch))
                        chunks[ci] = (obs_n, act_n, pr_n)
                obs_h, act_h, pr_h = chunks[ci]
                sc_h = t0 % CH
                ob0 = obs_h[0][:, sc_h:sc_h + 2]
                ob1 = obs_h[1][:, sc_h:sc_h + 2]
                pr2 = pr_h[:, sc_h:sc_h + 2]
                p_sm_pair = ppp.tile([128, 1024], F32, tag="sm_rzp",
                                     name=f"sm_rzp_{pi}")
                prs = (p_sm_pair[:, 0:512], p_sm_pair[:, 512:1024])
                for g in (0, 1):
                    psm = prs[g]
                    gsl = GS[g]
                    mm = nc.tensor.matmul
                    mm(psm, wsm_obs[:, 0][..., gsl], ob0, start=True, stop=False,
                       skip_group_check=True)
                    mm(psm, wsm_obs[:, 1][..., gsl], ob1, start=False, stop=False,
                       skip_group_check=True)
                    mm(psm, wsm_pr[:, gsl], pr2, start=False, stop=False,
                       skip_group_check=True)
                heads[pi] = (p_sm_pair, prs)

            emit_pair_head(0)
            for t0 in range(0, T_steps, 2):
                pi = t0 // 2
                ci = t0 // CH
                obs_c, act_c, pr_c = chunks[ci]
                p_sm_pair, prs_pair = heads.pop(pi)
                prs = {"sm": prs_pair}
                sc = t0 % CH

                for s in (0, 1):
                    t = t0 + s
                    sl = slice(s * 256, (s + 1) * 256)
                    obs_s = [obs_c[0][:, sc + s], obs_c[1][:, sc + s]]

                    # ---- small lane step ----
                    p_r, p_z = prs["sm"]
                    sm_sig_in = p_sm_pair.rearrange(
                        "p (g t b) -> p g t b", g=2, t=2)[:, :, s]
                    hh = wsm_hh
                    nc.tensor.matmul(p_r[:, sl], hh[:, GS[0]], h_sm,
                                     start=False, stop=(s == 1),
                                     skip_group_check=True)
                    nc.tensor.matmul(p_z[:, sl], hh[:, GS[1]], h_sm,
                                     start=False, stop=(s == 1),
                                     skip_group_check=True)
                    p_nh = pp.tile([128, 512], F32, tag="sm_nh", name=f"sm_nh_{t}")
                    nc.tensor.matmul(p_nh[:, 256:512], hh[:, GS[2]], h_sm,
                                     start=True, stop=True)
                    xn = [(wsm_obs[:, 0][..., GS[2]], obs_s[0]),
                          (wsm_obs[:, 1][..., GS[2]], obs_s[1]),
                          (wsm_pr[:, GS[2]], pr_c[:, sc + s])]
                    for i, (w, x) in enumerate(xn):
                        nc.tensor.matmul(p_nh[:, 0:256], w, x,
                                         start=(i == 0), stop=False,
                                         skip_group_check=True)
                    h_sm_prev = h_sm

                    # ---- oa lane matmuls (h_oa + x only) ----
                    p_rz_oa = pp2.tile([128, 512], F32, tag="oa_rz", name=f"oa_rz_{t}")
                    for g, psl in ((0, slice(0, 256)), (1, slice(256, 512))):
                        ihs = [(woa_obs[:, 0][..., GS[g]], obs_s[0]),
                               (woa_obs[:, 1][..., GS[g]], obs_s[1]),
                               (woa_act[:, GS[g]], act_c[:, sc + s]),
                               (woa_hh[:, GS[g]], h_oa)]
                        for i, (wt, x) in enumerate(ihs):
                            nc.tensor.matmul(p_rz_oa[:, psl], wt, x,
                                             start=(i == 0), stop=(i == 3))
                    p_nh_oa = pp.tile([128, 512], F32, tag="oa_nh", name=f"oa_nh_{t}")
                    nc.tensor.matmul(p_nh_oa[:, 256:512], woa_hh[:, GS[2]], h_oa,
                                     start=True, stop=True)
                    xn = [(woa_obs[:, 0][..., GS[2]], obs_s[0]),
                          (woa_obs[:, 1][..., GS[2]], obs_s[1]),
                          (woa_act[:, GS[2]], act_c[:, sc + s])]
                    for i, (w, x) in enumerate(xn):
                        nc.tensor.matmul(p_nh_oa[:, 0:256], w, x,
                                         start=(i == 0), stop=False,
                                         skip_group_check=True)

                    # ---- mx matmuls that need only h_mx ----
                    p_rz = pp.tile([128, 512], F32, tag="mx_rz", name=f"mx_rz_{t}")
                    for g, psl in ((0, slice(0, 256)), (1, slice(256, 512))):
                        nc.tensor.matmul(p_rz[:, psl], wmx_hh[:, GS[g]], h_mx,
                                         start=(g == 0), stop=False,
                                         skip_group_check=True)
                    p_nh2 = pp.tile([128, 512], F32, tag="mx_nh", name=f"mx_nh_{t}")
                    nc.tensor.matmul(p_nh2[:, 256:512], wmx_hh[:, GS[2]], h_mx,
                                     start=True, stop=True)

                    # ---- gate chains ----
                    h_sm = gru_elem("sm", t, sm_sig_in, p_z[:, sl], p_nh,
                                    h_sm, merged_sig=True)
                    h_oa = gru_elem("oa", t, p_rz_oa, p_rz_oa[:, 256:512],
                                    p_nh_oa, h_oa, merged_sig=True)

                    # ---- mx matmuls on fresh h_sm, then mx gates ----
                    for g, psl in ((0, slice(0, 256)), (1, slice(256, 512))):
                        nc.tensor.matmul(p_rz[:, psl], wmx_ih[:, GS[g]], h_sm,
                                         start=False, stop=(g == 1),
                                         skip_group_check=True)
                    nc.tensor.matmul(p_nh2[:, 0:256], wmx_ih[:, GS[2]], h_sm,
                                     start=True, stop=False,
                                     skip_group_check=True)
                    if s == 1 and t0 + 2 < T_steps:
                        emit_pair_head(t0 + 2)
                    h_mx = gru_elem("mx", t, p_rz, p_rz[:, 256:512],
                                    p_nh2, h_mx, merged_sig=True)

            # ---- value MLP on last states: feat = [h_oa; h_mx] ----
            w1t = cload("w1t")
            w2t = cload("w2t")
            h1 = []
            for m in range(2):
                p = pp.tile([128, BL], F32, tag=("oa_nh", "sm_nh")[m], name=f"p_h1_{m}")
                ms = slice(m * 128, (m + 1) * 128)
                nc.tensor.matmul(p, w1t[:, 0, ms], h_oa, start=True, stop=False)
                nc.tensor.matmul(p, w1t[:, 1, ms], h_mx, start=False, stop=True)
                h = tp.tile([128, BL], BF16, tag=f"h1_{m}", name=f"h1_{m}")
                bkw = ({"bias": bias["b1"][:, m:m + 1]} if use_bias else {})
                nc.scalar.activation(h, p, AFT.Tanh, **bkw)
                h1.append(h)
            p_val = pp.tile([1, BL], F32, tag="mx_rz", name="p_val")
            nc.tensor.matmul(p_val, w2t[:, 0], h1[0], start=True, stop=False)
            nc.tensor.matmul(p_val, w2t[:, 1], h1[1], start=False, stop=True)
            out_s = tp.tile([1, BL], F32, tag="out", name="out_s")
            if use_bias:
                nc.scalar.activation(out_s, p_val, AFT.Identity,
                                     bias=bias["b2"][0:1, 0:1])
            else:
                nc.scalar.activation(out_s, p_val, AFT.Identity)
            nc.sync.dma_start(val[:], out_s)

    nc.compile()
    return nc


def _prep_weights(inp: dict) -> dict:
    f4 = np.float32
    g = lambda w, i: np.asarray(w)[i * (w.shape[0] // 3):(i + 1) * (w.shape[0] // 3), :]
    out = {}
    out["woa_hh"] = np.ascontiguousarray(np.asarray(inp["oa_whh"]).T, f4)
    wih_oa_t = np.asarray(inp["oa_wih"]).T  # [320, 384]
    out["woa_obs"] = np.ascontiguousarray(
        wih_oa_t[0:256].reshape(2, 128, 384).transpose(1, 0, 2), f4)
    out["woa_act"] = np.ascontiguousarray(wih_oa_t[256:320], f4)
    out["wmx_hh"] = np.ascontiguousarray(np.asarray(inp["mx_whh"]).T, f4)
    perm = np.concatenate([np.arange(64, 128), np.arange(0, 32), np.arange(32, 64)])
    out["wmx_ih"] = np.ascontiguousarray(np.asarray(inp["mx_wih"]).T[perm], f4)

    wsm_hh = np.zeros((128, 384), f4)
    wsm_obs = np.zeros((256, 384), f4)
    wsm_pr = np.zeros((65, 384), f4)
    for gi in range(3):
        c = _GATE * gi
        wsm_hh[0:64, c + 0:c + 64] = g(inp["oo_whh"], gi).T
        wsm_hh[64:96, c + 64:c + 96] = g(inp["pa_whh"], gi).T
        wsm_hh[96:128, c + 96:c + 128] = g(inp["rr_whh"], gi).T
        wsm_obs[:, c + 0:c + 64] = g(inp["oo_wih"], gi).T
        wsm_pr[0:64, c + 64:c + 96] = g(inp["pa_wih"], gi).T
        wsm_pr[64:65, c + 96:c + 128] = g(inp["rr_wih"], gi).T
    out["wsm_hh"] = wsm_hh
    out["wsm_obs"] = np.ascontiguousarray(
        wsm_obs.reshape(2, 128, 384).transpose(1, 0, 2), f4)
    out["wsm_pr"] = wsm_pr

    out["w1t"] = np.ascontiguousarray(
        np.asarray(inp["W1"]).T.reshape(2, 128, 256).transpose(1, 0, 2), f4)
    out["w2t"] = np.ascontiguousarray(
        np.asarray(inp["W2"]).T.reshape(2, 128, 1).transpose(1, 0, 2), f4)
    out["wident"] = np.eye(128, dtype=f4)

    def pack_small(v_oo, v_pa, v_rr):
        r = np.zeros(128, f4)
        r[0:64], r[64:96], r[96:128] = v_oo, v_pa, v_rr
        return r

    for key, pfx in (("oa", "oa"), ("mx", "mx")):
        bih, bhh = np.asarray(inp[f"{key}_bih"]), np.asarray(inp[f"{key}_bhh"])
        H = bih.shape[0] // 3
        out[f"bnh_{pfx}"] = np.ascontiguousarray(bhh[2 * H:3 * H], f4).reshape(128, 1)
        out[f"bni_{pfx}"] = np.ascontiguousarray(bih[2 * H:3 * H], f4).reshape(128, 1)
        out[f"brz_{pfx}"] = np.ascontiguousarray(
            np.stack([bih[0:H] + bhh[0:H], bih[H:2 * H] + bhh[H:2 * H]], 1), f4)
    bsm = {}
    for part in ("bih", "bhh"):
        vs = {k: np.asarray(inp[f"{k}_{part}"]) for k in ("oo", "pa", "rr")}
        bsm[part] = [pack_small(vs["oo"][64 * gi:64 * (gi + 1)],
                                vs["pa"][32 * gi:32 * (gi + 1)],
                                vs["rr"][32 * gi:32 * (gi + 1)]) for gi in range(3)]
    out["bnh_sm"] = bsm["bhh"][2].reshape(128, 1)
    out["bni_sm"] = bsm["bih"][2].reshape(128, 1)
    out["brz_sm"] = np.ascontiguousarray(
        np.stack([bsm["bih"][0] + bsm["bhh"][0], bsm["bih"][1] + bsm["bhh"][1]], 1), f4)
    # b1 on-chip layout is [p, m] with feature index = m*128 + p
    out["b1"] = np.ascontiguousarray(np.asarray(inp["b1"]).reshape(2, 128).T, f4)
    out["b2"] = np.ascontiguousarray(np.asarray(inp["b2"]), f4).reshape(1, 1)
    return out


def _has_bias(w: dict) -> bool:
    return any(np.abs(w[name]).max() > 0 for name, _ in _BSPECS)


def _prep_core_inputs(inp: dict, w: dict, c: int, T_steps: int,
                      use_bias: bool):
    import ml_dtypes
    bf = ml_dtypes.bfloat16
    lay = _layout(T_steps)
    flat = np.empty(lay["_total"], bf)
    bs = slice(c * BL, (c + 1) * BL)

    nch = min(_CH, T_steps)
    ncks = (T_steps + nch - 1) // nch
    # obs: [ci][k(2)][128][nch][BL]
    v = flat[lay["obs"]:lay["obs"] + T_steps * F * BL].reshape(
        ncks, 2, 128, nch, BL)
    src_o = np.asarray(inp["obs"])[:T_steps, bs, :].reshape(
        ncks, nch, BL, 2, 128)
    np.copyto(v, src_o.transpose(0, 3, 4, 1, 2))
    # act: [ci][64][nch][BL]
    v = flat[lay["act"]:lay["act"] + T_steps * A * BL].reshape(
        ncks, A, nch, BL)
    src_a = np.asarray(inp["action"])[:T_steps, bs, :].reshape(
        ncks, nch, BL, A)
    np.copyto(v, src_a.transpose(0, 3, 1, 2))
    # pr: [ci][65][nch][BL]
    v = flat[lay["pr"]:lay["pr"] + T_steps * (A + 1) * BL].reshape(
        ncks, A + 1, nch, BL)
    src_p = np.asarray(inp["prev_action"])[:T_steps, bs, :].reshape(
        ncks, nch, BL, A)
    np.copyto(v[:, 0:A], src_p.transpose(0, 3, 1, 2))
    src_r = np.asarray(inp["reward"])[:T_steps, bs, :].reshape(
        ncks, nch, BL, 1)
    np.copyto(v[:, A:A + 1], src_r.transpose(0, 3, 1, 2))

    for name, shape in _WSPECS:
        sz = int(np.prod(shape))
        np.copyto(flat[lay[name]:lay[name] + sz].reshape(shape), w[name])

    m = {"flat": flat}
    if use_bias:
        blay = _blayout()
        extras = np.empty(blay["_total"], np.float32)
        for name, shape in _BSPECS:
            sz = int(np.prod(shape))
            np.copyto(extras[blay[name]:blay[name] + sz].reshape(shape),
                      w[name])
        m["extras"] = extras
    return m


_RUN_KW = {}
_CACHE = {}


def _get_nc(T_steps: int, use_bias: bool):
    from concourse.bass_interp import get_hw_module
    key = (T_steps, use_bias)
    if key not in _CACHE:
        nc = _build(T_steps, use_bias)
        nc.m = get_hw_module(nc.m)
        _CACHE[key] = nc
    return _CACHE[key]


def run(inputs: dict, T_steps: int = T, n_cores: int = NCORES):
    from concourse import bass_utils

    w = _prep_weights(inputs)
    use_bias = _has_bias(w)
    nc = _get_nc(T_steps, use_bias)
    in_maps = [_prep_core_inputs(inputs, w, c, T_steps, use_bias)
               for c in range(n_cores)]
    res = bass_utils.run_bass_kernel_spmd(
        nc, in_maps, core_ids=list(range(n_cores)), **_RUN_KW)
    vals = [res.results[c]["val"].reshape(BL) for c in range(n_cores)]
    out = np.concatenate(vals).astype(np.float32).reshape(-1, 1)
    run.last_result = res
    return out


def run_timed(inputs: dict, iters: int = 5, T_steps: int = T,
              n_cores: int = NCORES, pipeline: int = 16):
    """Wall-clocks repeated executions with device-resident inputs.
    Returns (out, seq_times, marginal_ns)."""
    import jax
    from concourse import bass2jax

    w = _prep_weights(inputs)
    use_bias = _has_bias(w)
    nc = _get_nc(T_steps, use_bias)
    in_maps = [_prep_core_inputs(inputs, w, c, T_steps, use_bias)
               for c in range(n_cores)]

    bass2jax.install_neuronx_cc_hook()
    partition_name = nc.partition_id_tensor.name if nc.partition_id_tensor else None
    in_names, out_names, out_avals, zero_outs = [], [], [], []
    import concourse.mybir as _my
    for alloc in nc.m.functions[0].allocations:
        if not isinstance(alloc, _my.MemoryLocationSet):
            continue
        name = alloc.memorylocations[0].name
        if alloc.kind == "ExternalInput":
            if name != partition_name:
                in_names.append(name)
        elif alloc.kind == "ExternalOutput":
            shape = tuple(alloc.tensor_shape)
            dtype = _my.dt.np(alloc.dtype)
            out_names.append(name)
            out_avals.append(jax.core.ShapedArray(shape, dtype))
            zero_outs.append(np.zeros(shape, dtype))
    n_params = len(in_names)
    all_in = list(in_names) + list(out_names)
    if partition_name is not None:
        all_in.append(partition_name)

    def _body(*args):
        operands = list(args)
        if partition_name is not None:
            operands.append(bass2jax.partition_id_tensor())
        outs = bass2jax._bass_exec_p.bind(
            *operands, out_avals=tuple(out_avals), in_names=tuple(all_in),
            out_names=tuple(out_names), lowering_input_output_aliases=(),
            sim_require_finite=True, sim_require_nnan=True, nc=nc)
        return tuple(outs)

    devices = jax.devices()[:n_cores]
    mesh = bass2jax.Mesh(np.asarray(devices), ("core",))
    donate = tuple(range(n_params, n_params + len(out_names)))
    sharded = jax.jit(
        bass2jax.shard_map(_body, mesh=mesh,
                           in_specs=(bass2jax.PartitionSpec("core"),) * (n_params + len(out_names)),
                           out_specs=(bass2jax.PartitionSpec("core"),) * len(out_names),
                           check_rep=False),
        donate_argnums=donate, keep_unused=True)

    concat_in = [np.concatenate([np.asarray(in_maps[c][nm]) for c in range(n_cores)], axis=0)
                 for nm in in_names]
    sh = jax.sharding.NamedSharding(mesh, bass2jax.PartitionSpec("core"))
    dev_in = [jax.device_put(x, sh) for x in concat_in]

    def zeros():
        return [jax.device_put(np.zeros((n_cores * z.shape[0], *z.shape[1:]), z.dtype), sh)
                for z in zero_outs]

    times = []
    out_arrs = None
    for _ in range(iters):
        zs = zeros()
        jax.block_until_ready(zs)
        t0 = time.time()
        out_arrs = sharded(*dev_in, *zs)
        jax.block_until_ready(out_arrs)
        times.append(time.time() - t0)

    marginal_ns = None
    if pipeline:
        tot = {}
        for N in (4, pipeline):
            best = 1e9
            for _ in range(2):
                zsets = [zeros() for _ in range(N)]
                jax.block_until_ready(zsets)
                t0 = time.time()
                outs = [sharded(*dev_in, *zs) for zs in zsets]
                jax.block_until_ready(outs)
                best = min(best, time.time() - t0)
            tot[N] = best
        marginal_ns = (tot[pipeline] - tot[4]) / (pipeline - 4) * 1e9

    res = {name: np.asarray(out_arrs[i]).reshape(n_cores, *out_avals[i].shape)
           for i, name in enumerate(out_names)}
    vals = [res["val"][c].reshape(BL) for c in range(n_cores)]
    out = np.concatenate(vals).astype(np.float32).reshape(-1, 1)
    return out, times, marginal_ns


def kernel(**inputs) -> np.ndarray:
    return run(inputs)
